# revision 1
# baseline (speedup 1.0000x reference)
"""Diagonal complex SSM (LRU-style scan) on 8 trn2 NeuronCores.

y[t,p,k] = Re( C @ s[t,:,k] ) + (D @ x[t,:,k])
s[t,n,k] = A[n,k] * s[t-1,n,k] + (B @ x[t,:,k])[n]     (complex, diagonal)

Strategy: shard K=32 across 8 cores (4 lanes each; B/C/D replicated, no
collectives). Per core, linearize the complex scan by phase:
    A = r * e^{i theta};  s_t = e^{i theta t} * sh_t
    sh_t = r * sh_{t-1} + e^{-i theta t} * (B x_t)
so the recurrence becomes two independent REAL first-order scans per lane
(hardware tensor_tensor_scan, fp32 state), with elementwise rotations by
host-precomputed cos/sin(theta*t mod 2pi) tables. The rotate-out adds are
folded into the C matmul using negated stationaries:
    y = Cre*(c.sre - s.sim) - Cim*(c.sim + s.sre)
      = Cre@p1 + (-Cre)@p2 + (-Cim)@p3 + (-Cim)@p4
"""

import numpy as np

from concourse import bacc, mybir
from concourse.tile import TileContext
from concourse.bass_utils import run_bass_kernel_spmd

T, N, U, K, P = 4096, 256, 128, 32, 128
NCORES = 8
KL = K // NCORES          # k-lanes per core
TB = 512                  # timesteps per block (1 PSUM bank @ fp32)
NT = T // TB
F32 = mybir.dt.float32
F32R = mybir.dt.float32r  # full-rate PE matmul dtype (fp32 bits)

MM_F32R = True           # use float32r for all matmuls
MMDT = F32R if MM_F32R else F32

_CACHE = {}


def _build():
    nc = bacc.Bacc("TRN2", target_bir_lowering=False, debug=False,
                   num_devices=NCORES)

    xT_d = nc.dram_tensor("xT", [U, KL, T], MMDT, kind="ExternalInput")
    cos_d = nc.dram_tensor("cosT", [N, KL, T], F32, kind="ExternalInput")
    sin_d = nc.dram_tensor("sinT", [N, KL, T], F32, kind="ExternalInput")
    # r packed [p, h*KL + k] so each (h, k) lane-column is a [128,1] slice
    r_d = nc.dram_tensor("rdec", [128, 2 * KL], F32, kind="ExternalInput")
    Bre_d = nc.dram_tensor("BTre", [U, N], MMDT, kind="ExternalInput")
    Bim_d = nc.dram_tensor("BTim", [U, N], MMDT, kind="ExternalInput")
    C1_d = nc.dram_tensor("CT1", [128, N], MMDT, kind="ExternalInput")  # +Cre^T
    C2_d = nc.dram_tensor("CT2", [128, N], MMDT, kind="ExternalInput")  # -Cre^T
    C3_d = nc.dram_tensor("CT3", [128, N], MMDT, kind="ExternalInput")  # -Cim^T
    DT_d = nc.dram_tensor("DT", [U, P], MMDT, kind="ExternalInput")
    y_d = nc.dram_tensor("yT", [P, KL, T], F32, kind="ExternalOutput")

    mult = mybir.AluOpType.mult
    add = mybir.AluOpType.add

    def mmcast(ap):
        return ap

    def b2(ap):
        # [128, TB] -> [128, 2, TB] stride-0 pair broadcast
        return ap.rearrange("p (one tb) -> p one tb",
                            one=1).broadcast_to([128, 2, TB])

    with TileContext(nc) as tc:
        with (
            tc.tile_pool(name="const", bufs=1) as cpool,
            tc.tile_pool(name="xp", bufs=3) as xpool,
            tc.tile_pool(name="tab", bufs=3) as tabpool,
            tc.tile_pool(name="wk", bufs=3) as wk,
            tc.tile_pool(name="pr", bufs=3) as prpool,
            tc.tile_pool(name="sh", bufs=2) as shpool,
            tc.tile_pool(name="yo", bufs=3) as ypool,
            tc.tile_pool(name="ups", bufs=1, space="PSUM") as upsum,
            tc.tile_pool(name="yps", bufs=2, space="PSUM") as ypsum,
        ):
            Bre = cpool.tile([U, N], MMDT)
            nc.sync.dma_start(Bre[:], Bre_d[:])
            Bim = cpool.tile([U, N], MMDT)
            nc.sync.dma_start(Bim[:], Bim_d[:])
            C1 = cpool.tile([128, N], MMDT)
            nc.sync.dma_start(C1[:], C1_d[:])
            C2 = cpool.tile([128, N], MMDT)
            nc.sync.dma_start(C2[:], C2_d[:])
            C3 = cpool.tile([128, N], MMDT)
            nc.sync.dma_start(C3[:], C3_d[:])
            DT = cpool.tile([U, P], MMDT)
            nc.sync.dma_start(DT[:], DT_d[:])
            rsb = cpool.tile([128, 2 * KL], F32)
            nc.sync.dma_start(rsb[:], r_d[:])

            prev = {}
            for tb in range(NT):
                t0 = tb * TB
                for k in range(KL):
                    xt = xpool.tile([U, TB], MMDT, tag="x")
                    nc.sync.dma_start(xt[:], xT_d[:, k, t0:t0 + TB])

                    prods = []  # (stationary_tile, h, product_tile)
                    for h in (0, 1):
                        hs = slice(h * 128, (h + 1) * 128)
                        # packed [cos | sin] table tile
                        cs3 = tabpool.tile([128, 2, TB], F32, tag=f"cs{h}")
                        nc.sync.dma_start(cs3[:, 0, :], cos_d[hs, k, t0:t0 + TB])
                        nc.sync.dma_start(cs3[:, 1, :], sin_d[hs, k, t0:t0 + TB])

                        u_re = upsum.tile([128, TB], F32, tag="ure")
                        u_im = upsum.tile([128, TB], F32, tag="uim")
                        nc.tensor.matmul(u_re[:], mmcast(Bre[:, hs]),
                                         mmcast(xt[:]), start=True, stop=True)
                        nc.tensor.matmul(u_im[:], mmcast(Bim[:, hs]),
                                         mmcast(xt[:]), start=True, stop=True)

                        # rotate-in paired by SHARED INPUT with
                        # ACT-derived table variants so even the complex
                        # adds pair:  pA = [c|-s]*u_re = [t1|-t4] (PSUM),
                        # pB = [s|c]*u_im = [t2|t3] (SBUF),
                        # uh2 = pB + pA = [uh_re | uh_im] in ONE add.
                        csm = tabpool.tile([128, 2, TB], F32, tag=f"csm{h}",
                                           bufs=2)
                        nc.scalar.copy(csm[:, 0, :], cs3[:, 0, :])
                        nc.scalar.mul(csm[:, 1, :], cs3[:, 1, :], -1.0)
                        ssc = tabpool.tile([128, 2, TB], F32, tag=f"ssc{h}",
                                           bufs=2)
                        nc.scalar.copy(ssc[:, 0, :], cs3[:, 1, :])
                        nc.scalar.copy(ssc[:, 1, :], cs3[:, 0, :])
                        pA = upsum.tile([128, 2 * TB], F32, tag="pA")
                        nc.vector.tensor_mul(
                            pA[:].rearrange("p (two tb) -> p two tb", two=2),
                            csm[:], b2(u_re[:]))
                        pB = wk.tile([128, 2, TB], F32, tag="pB")
                        nc.vector.tensor_mul(
                            pB[:], ssc[:], b2(u_im[:]))
                        uh2 = upsum.tile([128, 2 * TB], F32, tag="uh2")
                        nc.vector.tensor_add(
                            uh2[:].rearrange("p (two tb) -> p two tb", two=2),
                            pB[:],
                            pA[:].rearrange("p (two tb) -> p two tb", two=2))
                        uh_re = uh2[:, 0:TB]
                        uh_im = uh2[:, TB:2 * TB]

                        # hardware scans into a packed [im | re] tile.
                        # re on DVE (PSUM data1 keeps the shared SBUF port
                        # free); im on GpSimd.
                        ridx = h * KL + k
                        rb = rsb[:, ridx:ridx + 1].broadcast_to([128, TB])
                        sh2 = shpool.tile([128, 2 * TB], F32, tag=f"sh{k}{h}")
                        if tb == 0:
                            init_im, init_re = 0.0, 0.0
                        else:
                            pv = prev[(k, h)]
                            init_im = pv[:, TB - 1:TB]
                            init_re = pv[:, 2 * TB - 1:2 * TB]
                        nc.vector.tensor_tensor_scan(
                            sh2[:, 0:TB], rb, uh_im, init_im, mult, add)
                        nc.vector.tensor_tensor_scan(
                            sh2[:, TB:2 * TB], rb, uh_re, init_re,
                            mult, add)
                        prev[(k, h)] = sh2

                        # rotate-out products on GpSimd (concurrent with
                        # DVE thanks to PSUM-operand port discipline)
                        sh_im = sh2[:, 0:TB]
                        sh_re = sh2[:, TB:2 * TB]
                        # paired by shared input: [c|s]*sh_re = [p1|p4],
                        # [c|s]*sh_im = [p3|p2] (each waits on ONE scan)
                        ppr = prpool.tile([128, 2, TB], MMDT, tag=f"ppr{h}")
                        nc.gpsimd.tensor_mul(
                            ppr[:], cs3[:], b2(sh_re))
                        qpr = prpool.tile([128, 2, TB], MMDT, tag=f"qpr{h}")
                        nc.gpsimd.tensor_mul(
                            qpr[:], cs3[:], b2(sh_im))
                        prods += [(C1, h, ppr[:, 0, :]), (C2, h, qpr[:, 1, :]),
                                  (C3, h, qpr[:, 0, :]), (C3, h, ppr[:, 1, :])]

                    y_ps = ypsum.tile([P, TB], F32, tag="y")
                    nmm = len(prods) + 1
                    for i, (cst, h, pt) in enumerate(prods):
                        hs = slice(h * 128, (h + 1) * 128)
                        nc.tensor.matmul(y_ps[:], mmcast(cst[:, hs]),
                                         mmcast(pt),
                                         start=(i == 0), stop=False)
                    nc.tensor.matmul(y_ps[:], mmcast(DT[:]), mmcast(xt[:]),
                                     start=False, stop=True)

                    y_sb = ypool.tile([P, TB], F32, tag="ysb")
                    nc.scalar.copy(y_sb[:], y_ps[:])
                    nc.sync.dma_start(y_d[:, k, t0:t0 + TB], y_sb[:])

    nc.compile()
    return nc


def _host_prep(input_sequence, A_re, A_im, B_re, B_im, C_re, C_im, D):
    """Build the per-core input maps (numpy only)."""
    # Accept numpy or jax arrays.
    input_sequence = np.asarray(input_sequence, dtype=np.float32)
    A_re = np.asarray(A_re, dtype=np.float32)
    A_im = np.asarray(A_im, dtype=np.float32)
    B_re = np.asarray(B_re, dtype=np.float32)
    B_im = np.asarray(B_im, dtype=np.float32)
    C_re = np.asarray(C_re, dtype=np.float32)
    C_im = np.asarray(C_im, dtype=np.float32)
    D = np.asarray(D, dtype=np.float32)
    x = np.ascontiguousarray(input_sequence, dtype=np.float32)
    th = np.arctan2(A_im.astype(np.float64), A_re.astype(np.float64))  # (N,K)
    r = np.hypot(A_re.astype(np.float64), A_im.astype(np.float64))    # (N,K)

    t = np.arange(T, dtype=np.float64)
    # angle = theta * t  (mod 2pi), computed in fp64 then reduced
    ang = (th[:, :, None] * t[None, None, :]) % (2 * np.pi)  # (N, K, T)
    cosT = np.cos(ang).astype(np.float32)
    sinT = np.sin(ang).astype(np.float32)

    BTre = np.ascontiguousarray(B_re.T, dtype=np.float32)   # (U, N)
    BTim = np.ascontiguousarray(B_im.T, dtype=np.float32)
    CT1 = np.concatenate([C_re[:, :128].T, C_re[:, 128:].T], axis=1)
    CT2 = -CT1
    CT3 = np.concatenate([-C_im[:, :128].T, -C_im[:, 128:].T], axis=1)
    CT1 = np.ascontiguousarray(CT1, dtype=np.float32)       # (128, N)
    CT2 = np.ascontiguousarray(CT2, dtype=np.float32)
    CT3 = np.ascontiguousarray(CT3, dtype=np.float32)
    DT = np.ascontiguousarray(D.T, dtype=np.float32)        # (U, P)

    in_maps = []
    for c in range(NCORES):
        ks = slice(c * KL, (c + 1) * KL)
        xT = np.ascontiguousarray(x[:, :, ks].transpose(1, 2, 0))  # (U,KL,T)
        cosc = np.ascontiguousarray(cosT[:, ks, :])                # (N,KL,T)
        sinc = np.ascontiguousarray(sinT[:, ks, :])
        rc = r[:, ks].astype(np.float32)                           # (N,KL)
        # pack as [p, h*KL + k]
        rpk = np.concatenate([rc[:128, :], rc[128:, :]], axis=1)   # (128,2KL)
        rpk = np.ascontiguousarray(rpk, dtype=np.float32)
        in_maps.append(dict(xT=xT, cosT=cosc, sinT=sinc, rdec=rpk,
                            BTre=BTre, BTim=BTim, CT1=CT1, CT2=CT2,
                            CT3=CT3, DT=DT))
    return in_maps


def _get_nc():
    if "nc" not in _CACHE:
        _CACHE["nc"] = _build()
    return _CACHE["nc"]


def kernel(input_sequence, A_re, A_im, B_re, B_im, C_re, C_im, D,
           trace=False):
    nc = _get_nc()
    in_maps = _host_prep(input_sequence, A_re, A_im, B_re, B_im, C_re,
                         C_im, D)
    res = run_bass_kernel_spmd(nc, in_maps, core_ids=list(range(NCORES)),
                               trace=trace)
    out = np.empty((T, P, K), dtype=np.float32)
    for c in range(NCORES):
        yT = res.results[c]["yT"]                    # (P, KL, T)
        out[:, :, c * KL:(c + 1) * KL] = yT.transpose(2, 0, 1)
    if trace:
        _CACHE["exec_time_ns"] = res.exec_time_ns
    return out



# revision 2
# speedup vs baseline: 2.1666x; 2.1666x over previous
"""Diagonal complex SSM (LRU-style scan) on 8 trn2 NeuronCores.

y[t,p,k] = Re( C @ s[t,:,k] ) + (D @ x[t,:,k])
s[t,n,k] = A[n,k] * s[t-1,n,k] + (B @ x[t,:,k])[n]     (complex, diagonal)

Strategy: shard K=32 across 8 cores (4 lanes each, no collectives), and
decimate time by M=4 so the sequential scan runs on the lattice t_L=4L+3
only (T'=1024 per lane).  The decimation folds into precomputed matmul
stationaries:
  U_L    = sum_d (diag(a^d) B) @ x[4L+3-d]          (lattice input)
  S_L    = a^4 S_{L-1} + U_L                         (lattice scan)
  y[4L+c]= Re(C diag(a^{c+1})) @ ReIm(S_{L-1})       (recovery)
         + sum_{d<=c} Re(C diag(a^d) B + [d==0]D) @ x[4L+c-d]   (conv)
The complex lattice scan is phase-linearized (a^4 = r^4 e^{i4th}):
S_L = e^{i phi_L} Z_L with phi_L = th*t_L, giving two REAL hardware
scans per lane with decay r^4, plus elementwise rotations by
host-precomputed bf16 cos/sin tables packed as 5 windows
[-s | c | s | c | -s] so every rotation variant is a view.
Everything except the scan state is bf16 (4x matmul, 2x DVE).
"""

import numpy as np
import ml_dtypes

from concourse import bacc, mybir
from concourse.tile import TileContext
from concourse.bass_utils import run_bass_kernel_spmd

T, N, U, K, P = 4096, 256, 128, 32, 128
NCORES = 8
KL = K // NCORES          # k-lanes per core
M = 4                     # time decimation
Tp = T // M               # lattice length (1024)
TB = 512                  # lattice points per chunk
NT = Tp // TB             # chunks (2)
F32 = mybir.dt.float32
BF16 = mybir.dt.bfloat16
BF = ml_dtypes.bfloat16

_CACHE = {}


def _build():
    nc = bacc.Bacc("TRN2", target_bir_lowering=False, debug=False,
                   num_devices=NCORES)

    xb_d = nc.dram_tensor("xb", [U, KL, M, Tp], BF16, kind="ExternalInput")
    tab_d = nc.dram_tensor("tab5", [N, KL, 5, Tp], BF16, kind="ExternalInput")
    r4_d = nc.dram_tensor("r4", [128, 2 * KL], F32, kind="ExternalInput")
    # stationaries pre-laid in SBUF layout (one identity DMA each)
    Bst_d = nc.dram_tensor("Bst", [U, KL * 16 * 128], BF16,
                           kind="ExternalInput")
    Wst_d = nc.dram_tensor("Wst", [128, KL * 16 * 128], BF16,
                           kind="ExternalInput")
    Cst_d = nc.dram_tensor("Cst", [U, KL * M * 128], BF16,
                           kind="ExternalInput")
    yb_d = nc.dram_tensor("yb", [P, KL, M, Tp], F32, kind="ExternalOutput")

    mult = mybir.AluOpType.mult
    add = mybir.AluOpType.add

    def b2(ap):
        # [128, TB] -> [128, 2, TB] stride-0 pair broadcast
        return ap.rearrange("p (one tb) -> p one tb",
                            one=1).broadcast_to([128, 2, TB])

    with TileContext(nc) as tc:
        with (
            tc.tile_pool(name="const", bufs=1) as cpool,
            tc.tile_pool(name="xp", bufs=3) as xpool,
            tc.tile_pool(name="tab", bufs=3) as tabpool,
            tc.tile_pool(name="u2", bufs=2) as u2pool,
            tc.tile_pool(name="uh", bufs=2) as uhpool,
            tc.tile_pool(name="sh", bufs=2) as shpool,
            tc.tile_pool(name="w", bufs=2) as wpool,
            tc.tile_pool(name="yo", bufs=3) as ypool,
            tc.tile_pool(name="ups", bufs=1, space="PSUM") as upsum,
            tc.tile_pool(name="yps", bufs=1, space="PSUM") as ypsum,
        ):
            Bsb = cpool.tile([U, KL * 16 * 128], BF16)
            nc.sync.dma_start(Bsb[:], Bst_d[:])
            Wsb = cpool.tile([128, KL * 16 * 128], BF16)
            nc.sync.dma_start(Wsb[:], Wst_d[:])
            Csb = cpool.tile([U, KL * M * 128], BF16)
            nc.sync.dma_start(Csb[:], Cst_d[:])
            r4sb = cpool.tile([128, 2 * KL], F32)
            nc.sync.dma_start(r4sb[:], r4_d[:])

            def bslice(k, d, ri, h):
                i = ((k * 4 + d) * 2 + ri) * 2 + h
                return Bsb[:, i * 128:(i + 1) * 128]

            def wslice(k, c, ri, h):
                i = ((k * 4 + c) * 2 + ri) * 2 + h
                return Wsb[:, i * 128:(i + 1) * 128]

            def cslice(k, d):
                i = k * M + d
                return Csb[:, i * 128:(i + 1) * 128]

            prev = {}
            for tb in range(NT):
                L0 = tb * TB
                for k in range(KL):
                    xt = xpool.tile([U, M, TB], BF16, tag="x")
                    nc.sync.dma_start(xt[:], xb_d[:, k, :, L0:L0 + TB])

                    wexts = []
                    for h in (0, 1):
                        hs = slice(h * 128, (h + 1) * 128)
                        tab = tabpool.tile([128, 5, TB], BF16, tag=f"t{h}")
                        nc.sync.dma_start(tab[:], tab_d[hs, k, :, L0:L0 + TB])

                        # lattice-B: U_L = sum_d Bd @ x[:, M-1-d, :]
                        u_re = upsum.tile([128, TB], F32, tag="ure")
                        u_im = upsum.tile([128, TB], F32, tag="uim")
                        for d in range(M):
                            nc.tensor.matmul(u_re[:], bslice(k, d, 0, h),
                                             xt[:, M - 1 - d, :],
                                             start=(d == 0), stop=(d == M - 1))
                        for d in range(M):
                            nc.tensor.matmul(u_im[:], bslice(k, d, 1, h),
                                             xt[:, M - 1 - d, :],
                                             start=(d == 0), stop=(d == M - 1))
                        u2 = u2pool.tile([128, 2, TB], BF16, tag="u2")
                        nc.scalar.copy(u2[:, 0, :], u_re[:])
                        nc.scalar.copy(u2[:, 1, :], u_im[:])

                        # rotate-in: V = e^{-i phi} U
                        # pA = [c|-s]*[ure|ure], pB = [s|c]*[uim|uim]
                        # uh = pA+pB = [V_re | V_im]
                        pA = uhpool.tile([128, 2, TB], BF16, tag="pA")
                        nc.vector.tensor_mul(pA[:], tab[:, 3:5, :],
                                             b2(u2[:, 0, :]))
                        pB = uhpool.tile([128, 2, TB], BF16, tag="pB")
                        nc.vector.tensor_mul(pB[:], tab[:, 2:4, :],
                                             b2(u2[:, 1, :]))
                        uh = uhpool.tile([128, 2, TB], BF16, tag="uh")
                        nc.vector.tensor_add(uh[:], pA[:], pB[:])

                        # lattice scans: Z = scan(r^4, V)
                        ridx = h * KL + k
                        rb = r4sb[:, ridx:ridx + 1].broadcast_to([128, TB])
                        sh2 = shpool.tile([128, 2 * TB], BF16, tag=f"sh{k}{h}")
                        if tb == 0:
                            init_re, init_im = 0.0, 0.0
                        else:
                            pv = prev[(k, h)]
                            init_re = pv[:, TB - 1:TB]
                            init_im = pv[:, 2 * TB - 1:2 * TB]
                        nc.vector.tensor_tensor_scan(
                            sh2[:, 0:TB], rb, uh[:, 0, :], init_re, mult, add)
                        nc.vector.tensor_tensor_scan(
                            sh2[:, TB:2 * TB], rb, uh[:, 1, :], init_im,
                            mult, add)
                        prev[(k, h)] = sh2

                        # rotate-out: w = S_L = e^{+i phi} Z, shifted by one
                        # lattice col into wext (col 0 = S_{L0-1})
                        wext = wpool.tile([128, 2, TB + 1], BF16,
                                          tag=f"w{k}{h}")
                        if tb == 0:
                            nc.vector.memset(wext[:, :, 0:1], 0.0)
                        else:
                            nc.scalar.copy(wext[:, :, 0:1],
                                           wprev[(k, h)][:, :, TB:TB + 1])
                        pAo = uhpool.tile([128, 2, TB], BF16, tag=f"pAo{h}")
                        nc.gpsimd.tensor_mul(pAo[:], tab[:, 1:3, :],
                                             b2(sh2[:, 0:TB]))
                        pBo = uhpool.tile([128, 2, TB], BF16, tag=f"pBo{h}")
                        nc.gpsimd.tensor_mul(pBo[:], tab[:, 0:2, :],
                                             b2(sh2[:, TB:2 * TB]))
                        nc.vector.tensor_add(wext[:, :, 1:TB + 1],
                                             pAo[:], pBo[:])
                        wexts.append(wext)

                    if tb == 0 and k == 0:
                        wprev = {}
                    for h in (0, 1):
                        wprev[(k, h)] = wexts[h]

                    # y assembly per offset class c
                    for c in range(M):
                        y_ps = ypsum.tile([P, TB], F32, tag=f"y{c}")
                        nmm = 0
                        total = 4 + (c + 1)
                        for h in (0, 1):
                            for ri in (0, 1):
                                nc.tensor.matmul(
                                    y_ps[:], wslice(k, c, ri, h),
                                    wexts[h][:, ri, 0:TB],
                                    start=(nmm == 0), stop=(nmm == total - 1))
                                nmm += 1
                        for d in range(c + 1):
                            nc.tensor.matmul(
                                y_ps[:], cslice(k, d), xt[:, c - d, :],
                                start=(nmm == 0), stop=(nmm == total - 1))
                            nmm += 1
                        y_sb = ypool.tile([P, TB], F32, tag="ysb")
                        nc.scalar.copy(y_sb[:], y_ps[:])
                        nc.sync.dma_start(yb_d[:, k, c, L0:L0 + TB], y_sb[:])

    nc.compile()
    return nc


def _host_prep(input_sequence, A_re, A_im, B_re, B_im, C_re, C_im, D):
    """Build the per-core input maps (numpy only)."""
    x = np.ascontiguousarray(np.asarray(input_sequence, dtype=np.float32))
    A = (np.asarray(A_re, np.float64) + 1j * np.asarray(A_im, np.float64))
    Bm = (np.asarray(B_re, np.float64) + 1j * np.asarray(B_im, np.float64))
    Cm = (np.asarray(C_re, np.float64) + 1j * np.asarray(C_im, np.float64))
    Dm = np.asarray(D, np.float64)

    r = np.abs(A)                       # (N, K)
    th = np.angle(A)
    r4 = (r ** M).astype(np.float32)

    # lattice phase tables, 5 windows [-s | c | s | c | -s]
    tL = (M * np.arange(Tp) + M - 1).astype(np.float64)
    ang = (th[:, :, None] * tL[None, None, :]) % (2 * np.pi)  # (N, K, Tp)
    c_t = np.cos(ang)
    s_t = np.sin(ang)
    tab5 = np.stack([-s_t, c_t, s_t, c_t, -s_t], axis=2)  # (N, K, 5, Tp)
    tab5 = tab5.astype(BF)

    # x blocked: xb[u, k, c, L] = x[4L+c, u, k]
    xb = x.transpose(1, 2, 0).reshape(U, K, Tp, M).transpose(0, 1, 3, 2)
    xb = np.ascontiguousarray(xb).astype(BF)

    # stationaries per k
    dpow = np.arange(M)
    ak = A.T                                  # (K, N)
    Bd = (ak[:, None, :, None] ** dpow[None, :, None, None]) \
        * Bm[None, None, :, :]                # (K, M, N, U) = diag(a^d) B
    Wc = Cm[None, None, :, :] * (ak[:, None, None, :]
                                 ** np.arange(1, M + 1)[None, :, None, None])
    Dc = np.real(np.einsum('pn,kdn,nu->kdpu',
                           Cm, ak[:, None, :] ** dpow[None, :, None], Bm))
    Dc[:, 0] += Dm[None, :, :]

    in_maps = []
    for core in range(NCORES):
        ks = slice(core * KL, (core + 1) * KL)
        kk = range(core * KL, (core + 1) * KL)

        Bst = np.empty((U, KL * 16 * 128), np.float32)
        Wst = np.empty((128, KL * 16 * 128), np.float32)
        Cst = np.empty((U, KL * M * 128), np.float32)
        for ki, kg in enumerate(kk):
            for d in range(M):
                for ri in range(2):
                    part = np.real(Bd[kg, d]) if ri == 0 else np.imag(Bd[kg, d])
                    for h in range(2):
                        i = ((ki * 4 + d) * 2 + ri) * 2 + h
                        # lhsT [U, 128]: stat.T of rows h*128:(h+1)*128
                        Bst[:, i * 128:(i + 1) * 128] = \
                            part[h * 128:(h + 1) * 128, :].T
            for c in range(M):
                for ri in range(2):
                    part = np.real(Wc[kg, c]) if ri == 0 \
                        else -np.imag(Wc[kg, c])
                    for h in range(2):
                        i = ((ki * 4 + c) * 2 + ri) * 2 + h
                        # lhsT [n-half, P]
                        Wst[:, i * 128:(i + 1) * 128] = \
                            part[:, h * 128:(h + 1) * 128].T
            for d in range(M):
                i = ki * M + d
                Cst[:, i * 128:(i + 1) * 128] = Dc[kg, d].T

        rc = r4[:, ks]                                   # (N, KL)
        rpk = np.concatenate([rc[:128, :], rc[128:, :]], axis=1)
        in_maps.append(dict(
            xb=np.ascontiguousarray(xb[:, ks]),
            tab5=np.ascontiguousarray(tab5[:, ks]),
            r4=np.ascontiguousarray(rpk, np.float32),
            Bst=Bst.astype(BF), Wst=Wst.astype(BF), Cst=Cst.astype(BF),
        ))
    return in_maps


def _get_nc():
    if "nc" not in _CACHE:
        _CACHE["nc"] = _build()
    return _CACHE["nc"]


def kernel(input_sequence, A_re, A_im, B_re, B_im, C_re, C_im, D,
           trace=False):
    nc = _get_nc()
    in_maps = _host_prep(input_sequence, A_re, A_im, B_re, B_im, C_re,
                         C_im, D)
    res = run_bass_kernel_spmd(nc, in_maps, core_ids=list(range(NCORES)),
                               trace=trace)
    out = np.empty((T, P, K), dtype=np.float32)
    for c in range(NCORES):
        yb = res.results[c]["yb"]                    # (P, KL, M, Tp)
        # out[4L+m, p, k] = yb[p, k, m, L]
        y = yb.transpose(3, 2, 0, 1).reshape(T, P, KL)
        out[:, :, c * KL:(c + 1) * KL] = y
    if trace:
        _CACHE["exec_time_ns"] = res.exec_time_ns
    return out
